# revision 15
# baseline (speedup 1.0000x reference)
"""Trainium2 Bass kernel for nn_GATmGCN (2-layer 8-head GAT + 2-layer weighted GCN,
output = elementwise max of the branches). 8-core SPMD, nodes sharded by dst range.

Design:
 - Host: sort edges by dst, shard nodes into contiguous per-core chunks, pack each
   core's edges into fixed tiles of ND=32 dst nodes with 3 A-table + 3 B-table
   128-slot gather blocks (node table split in half so int16 dma_gather indices fit).
 - Device per tile: dma_gather source rows [x|es] (fp16), one-hot matmuls accumulate
   unnormalized per-(head,node) sums + softmax denominators in PSUM (feature-major),
   normalization = reciprocal + ones-rank-1-matmul broadcast + multiply, then the
   weight matrices are applied at node level (aggregation/weight-mix commute).
 - Attention logits: es[src] rides in gathered rows; ed[dst] via one-hotT matmuls.
 - Layer 0 -> layer 1 node features exchanged with an AllGather.
"""
import numpy as np

_NC_CACHE = {}


def _cfg(N=50000, E=800000, n_cores=8):
    D, H, ND, ABLK, BBLK = 128, 8, 32, 3, 3
    SB = ABLK + BBLK
    t_global = (N + ND - 1) // ND
    T = (t_global + n_cores - 1) // n_cores          # tiles per core
    NPCP = T * ND                                     # padded nodes per core
    NPAD = NPCP * n_cores
    return dict(N=N, E=E, D=D, H=H, NC=n_cores, ND=ND, ABLK=ABLK, BBLK=BBLK,
                SB=SB, T=T, NPCP=NPCP, NPAD=NPAD, NHALF=NPAD // 2,
                ROW0=256, ROW1=384, ESO0=128, ESO1=256)


# ------------------------------------------------------------------ host side
def _wrap16(idx_flat):
    """[n] -> wrapped [16, n//16] (idx[i] = arr[i%16, i//16]) replicated to 128."""
    n = len(idx_flat)
    arr = idx_flat.astype(np.int16).reshape(n // 16, 16).T
    return np.tile(arr, (8, 1))


def _preprocess(cfg, edge_index, edge_weight):
    ND, ABLK, BBLK, SB = cfg["ND"], cfg["ABLK"], cfg["BBLK"], cfg["SB"]
    T, NPCP, NHALF, H = cfg["T"], cfg["NPCP"], cfg["NHALF"], cfg["H"]
    CAP_A, CAP_B = ABLK * 128, BBLK * 128

    src = np.asarray(edge_index[0], dtype=np.int64)
    dst = np.asarray(edge_index[1], dtype=np.int64)
    w = np.asarray(edge_weight, dtype=np.float32)
    order = np.argsort(dst, kind="stable")
    src, dst, w = src[order], dst[order], w[order]

    cores = []
    for c in range(cfg["NC"]):
        lo = c * NPCP
        a = np.searchsorted(dst, lo, "left")
        b = np.searchsorted(dst, lo + NPCP, "left")
        s_c, d_c, w_c = src[a:b], dst[a:b] - lo, w[a:b]
        tile = d_c // ND
        ldst = d_c - tile * ND
        isB = (s_c >= NHALF).astype(np.int64)
        # stable order by (tile, isB); edges already tile-sorted
        o2 = np.lexsort((np.arange(len(s_c)), isB, tile))
        tile, ldst, isB = tile[o2], ldst[o2], isB[o2]
        s2, w2 = s_c[o2], w_c[o2]
        # rank within (tile, isB) group
        gkey = tile * 2 + isB
        if len(gkey):
            starts = np.r_[0, np.flatnonzero(np.diff(gkey)) + 1]
            gid = np.zeros(len(gkey), np.int64)
            gid[starts[1:]] = 1
            gid = np.cumsum(gid)
            rank = np.arange(len(gkey)) - starts[gid]
            cnts = np.bincount(gkey, minlength=2 * T)
            assert cnts[0::2].max(initial=0) <= CAP_A, "A-run overflow"
            assert cnts[1::2].max(initial=0) <= CAP_B, "B-run overflow"
        else:
            rank = gkey
        slot = np.where(isB == 0, rank, CAP_A + rank)
        blk, part = slot // 128, slot % 128

        # wrapped gather indices (pad slots -> row 0)
        sidxA = np.zeros((T, CAP_A), np.int64)
        sidxB = np.zeros((T, CAP_B), np.int64)
        selA = isB == 0
        sidxA[tile[selA], rank[selA]] = s2[selA]
        sidxB[tile[~selA], rank[~selA]] = s2[~selA] - NHALF
        idx_s = np.zeros((T, 128, SB * 8), np.int16)
        wrapA = sidxA.reshape(T, CAP_A // 16, 16).transpose(0, 2, 1).astype(np.int16)
        wrapB = sidxB.reshape(T, CAP_B // 16, 16).transpose(0, 2, 1).astype(np.int16)
        idx_s[:, :, :ABLK * 8] = np.tile(wrapA, (1, 8, 1))
        idx_s[:, :, ABLK * 8:] = np.tile(wrapB, (1, 8, 1))

        oh_s = np.zeros((T, 128, SB * ND), np.float16)
        oh_s[tile, part, blk * ND + ldst] = 1.0
        ohT_s = np.zeros((T, ND, SB * 128), np.float16)
        ohT_s[tile, ldst, blk * 128 + part] = 1.0
        w_s = np.zeros((T, 128, SB), np.float16)
        w_s[tile, part, blk] = w2
        cores.append(dict(idx=idx_s, oh=oh_s, ohT=ohT_s, w=w_s))
    return cores


def _host_weights(cfg, x, gat_W0, gat_a0_src, gat_a0_dst, gat_W1, gat_a1_src,
                  gat_a1_dst, gcn_W0, gcn_W1):
    N, D, H, NPAD = cfg["N"], cfg["D"], cfg["H"], cfg["NPAD"]
    F0 = gat_W0.shape[2]
    B0src = np.einsum("hif,hf->ih", gat_W0, gat_a0_src).astype(np.float32)
    B0dst = np.einsum("hif,hf->ih", gat_W0, gat_a0_dst).astype(np.float32)
    B1src = np.einsum("hif,hf->ih", gat_W1, gat_a1_src).astype(np.float32)
    B1dst = np.einsum("hif,hf->ih", gat_W1, gat_a1_dst).astype(np.float32)
    es0 = x @ B0src
    ed0 = x @ B0dst
    W0cat = gat_W0.transpose(1, 0, 2).reshape(D, H * F0).astype(np.float32)
    W1cat = gat_W1.transpose(1, 0, 2).reshape(D, H * D).astype(np.float32)
    table0 = np.zeros((NPAD, cfg["ROW0"]), np.float16)
    table0[:N, :D] = x.astype(np.float16)
    table0[:N, cfg["ESO0"]:cfg["ESO0"] + H] = es0.astype(np.float16)
    return dict(table0=table0, ed0=ed0.astype(np.float16), B1src=B1src,
                B1dst=B1dst, W0cat=W0cat, W1cat=W1cat,
                gcn_W0=gcn_W0.astype(np.float32), gcn_W1=gcn_W1.astype(np.float32))


# ------------------------------------------------------------------ device side
def _build(cfg, reps=1, qrot=4):
    import concourse.bass as bass
    import concourse.tile as tile
    from concourse import bacc, mybir
    from concourse.masks import make_identity
    from contextlib import ExitStack

    f16, f32 = mybir.dt.float16, mybir.dt.float32
    i16 = mybir.dt.int16
    OP = mybir.AluOpType
    AF = mybir.ActivationFunctionType
    D, H, ND, SB, ABLK, BBLK = (cfg[k] for k in ("D", "H", "ND", "SB", "ABLK", "BBLK"))
    T, NPCP, NPAD, NHALF = (cfg[k] for k in ("T", "NPCP", "NPAD", "NHALF"))
    ROW0, ROW1, ESO0, ESO1 = (cfg[k] for k in ("ROW0", "ROW1", "ESO0", "ESO1"))
    NC = cfg["NC"]
    C9 = 9 * ND              # channels per sub-block in S (8 GAT + 1 GCN)
    CG = 8 * ND              # GAT channel count

    nc = bacc.Bacc("TRN2", target_bir_lowering=False, debug=False,
                   num_devices=NC, num_swdge_queues=4)

    t0 = nc.dram_tensor("table0", [NPAD, ROW0], f16, kind="ExternalInput").ap()
    idx_s = nc.dram_tensor("idx_s", [T, 128, SB * 8], i16, kind="ExternalInput").ap()
    oh_s = nc.dram_tensor("oh_s", [T, 128, SB * ND], f16, kind="ExternalInput").ap()
    ohT_s = nc.dram_tensor("ohT_s", [T, ND, SB * 128], f16, kind="ExternalInput").ap()
    w_s = nc.dram_tensor("w_s", [T, 128, SB], f16, kind="ExternalInput").ap()
    ed0_d = nc.dram_tensor("ed0", [NPCP, H], f16, kind="ExternalInput").ap()
    b1s_d = nc.dram_tensor("B1src", [D, H], f32, kind="ExternalInput").ap()
    b1d_d = nc.dram_tensor("B1dst", [D, H], f32, kind="ExternalInput").ap()
    w0c_d = nc.dram_tensor("W0cat", [D, D], f32, kind="ExternalInput").ap()
    w1c_d = nc.dram_tensor("W1cat", [D, H * D], f32, kind="ExternalInput").ap()
    gw0_d = nc.dram_tensor("gcnW0", [D, D], f32, kind="ExternalInput").ap()
    gw1_d = nc.dram_tensor("gcnW1", [D, D], f32, kind="ExternalInput").ap()
    out_d = nc.dram_tensor("out", [NPCP, D], f32, kind="ExternalOutput").ap()

    t1c = nc.dram_tensor("t1_chunk", [NPCP, ROW1], f16).ap()
    t1f = nc.dram_tensor("t1_full", [NPAD, ROW1], f16, addr_space="Shared").ap()
    ed1_d = nc.dram_tensor("ed1", [NPCP, H], f16).ap()

    with tile.TileContext(nc) as tc, ExitStack() as ctx, \
            nc.allow_low_precision(reason="fp16 edge pipeline by design"):
        const = ctx.enter_context(tc.tile_pool(name="const", bufs=1))
        ip = ctx.enter_context(tc.tile_pool(name="ip", bufs=2))
        gp = ctx.enter_context(tc.tile_pool(name="gp", bufs=3))
        hp = ctx.enter_context(tc.tile_pool(name="hp", bufs=2))
        sp = ctx.enter_context(tc.tile_pool(name="sp", bufs=2))
        yp = ctx.enter_context(tc.tile_pool(name="yp", bufs=2))
        fp = ctx.enter_context(tc.tile_pool(name="fp", bufs=2))
        pp = ctx.enter_context(tc.tile_pool(name="pp", bufs=1, space="PSUM"))
        p_ed = p_q = p_s = p_rb = p_m = p_tr = pp

        # constants
        ones_col = const.tile([128, 1], f16)
        nc.vector.memset(ones_col[:], 1.0)
        ones_row = const.tile([1, 128], f16)
        nc.vector.memset(ones_row[:], 1.0)
        idn16 = const.tile([128, 128], f16)
        make_identity(nc, idn16[:])
        idn32 = const.tile([128, 128], f32)
        make_identity(nc, idn32[:])
        w0c_t = const.tile([128, D], f16)
        nc.gpsimd.dma_start(w0c_t[:], w0c_d[:, :])
        w1c_t = const.tile([128, H * D], f16)
        nc.gpsimd.dma_start(w1c_t[:], w1c_d[:, :])
        gw0_t = const.tile([128, D], f16)
        nc.gpsimd.dma_start(gw0_t[:], gw0_d[:, :])
        gw1_t = const.tile([128, D], f16)
        nc.gpsimd.dma_start(gw1_t[:], gw1_d[:, :])
        b1s_t = const.tile([128, H], f16)
        nc.gpsimd.dma_start(b1s_t[:], b1s_d[:, :])
        b1d_t = const.tile([128, H], f16)
        nc.gpsimd.dma_start(b1d_t[:], b1d_d[:, :])

        for rep in range(reps):
            for layer in (0, 1):
                ROW = ROW0 if layer == 0 else ROW1
                ESO = ESO0 if layer == 0 else ESO1
                table = t0 if layer == 0 else t1f
                ed_src = ed0_d if layer == 0 else ed1_d

                for t in range(T):
                    base = t * ND
                    # ---- inputs for this tile
                    idx_t = ip.tile([128, SB * 8], i16, tag="idx")
                    nc.sync.dma_start(idx_t[:], idx_s[t])
                    oh_t = hp.tile([128, SB * ND], f16, tag="oh")
                    nc.sync.dma_start(oh_t[:], oh_s[t])
                    ohT_t = hp.tile([ND, SB * 128], f16, tag="ohT")
                    nc.sync.dma_start(ohT_t[:], ohT_s[t])
                    w_t = hp.tile([128, SB], f16, tag="w")
                    nc.sync.dma_start(w_t[:], w_s[t])
                    edT_t = hp.tile([ND, H], f16, tag="edT")
                    nc.sync.dma_start(edT_t[:], ed_src[base:base + ND, :])

                    # ---- gathers (A blocks then B blocks)
                    g_t = gp.tile([128, SB * ROW], f16, tag="g")
                    nc.gpsimd.dma_gather(
                        out_ap=g_t[:, :ABLK * ROW].rearrange("p (n e) -> p n e", e=ROW),
                        in_ap=table[0:NHALF, :],
                        idxs_ap=idx_t[:, :ABLK * 8],
                        num_idxs=ABLK * 128, num_idxs_reg=ABLK * 128,
                        elem_size=ROW, single_packet=False, queue_num=(2 * t) % qrot)
                    nc.gpsimd.dma_gather(
                        out_ap=g_t[:, ABLK * ROW:].rearrange("p (n e) -> p n e", e=ROW),
                        in_ap=table[NHALF:2 * NHALF, :],
                        idxs_ap=idx_t[:, ABLK * 8:],
                        num_idxs=BBLK * 128, num_idxs_reg=BBLK * 128,
                        elem_size=ROW, single_packet=False, queue_num=(2 * t + 1) % qrot)

                    # ---- attention logits z = exp(lrelu(es[src] + ed[dst]))
                    ped = p_ed.tile([128, SB * H], f32, tag="ped")
                    for b in range(SB):
                        nc.tensor.matmul(
                            out=ped[:, b * H:(b + 1) * H],
                            lhsT=ohT_t[:, b * 128:(b + 1) * 128],
                            rhs=edT_t[:], start=True, stop=True)
                    es_view = g_t[:].rearrange("p (b e) -> p b e", e=ROW)[:, :, ESO:ESO + H]
                    u_t = sp.tile([128, SB * H], f16, tag="u")
                    nc.vector.tensor_tensor(
                        out=u_t[:], in0=ped[:], in1=es_view, op=OP.add)
                    lr_t = sp.tile([128, SB * H], f16, tag="lr")
                    nc.vector.tensor_scalar_mul(lr_t[:], u_t[:], 0.2)
                    nc.vector.tensor_tensor(out=lr_t[:], in0=u_t[:], in1=lr_t[:],
                                            op=OP.max)
                    attw = sp.tile([128, SB * 9], f16, tag="attw")
                    av = attw[:].rearrange("p (b c) -> p b c", c=9)
                    nc.scalar.activation(
                        av[:, :, 0:H],
                        lr_t[:].rearrange("p (b c) -> p b c", c=H), AF.Exp)
                    nc.vector.tensor_copy(av[:, :, H:9], w_t[:, :, None])

                    # ---- S = attw (x) onehot   [128, SB*9*ND]
                    s_t = sp.tile([128, SB * C9], f16, tag="s")
                    sv = s_t[:].rearrange("p (b c j) -> p b c j", c=9, j=ND)
                    ohv = oh_t[:].rearrange("p (b j) -> p b j", j=ND)
                    nc.vector.tensor_tensor(
                        out=sv,
                        in0=ohv[:, :, None, :].to_broadcast((128, SB, 9, ND)),
                        in1=av[:, :, :, None].to_broadcast((128, SB, 9, ND)),
                        op=OP.mult)

                    # ---- aggregation matmuls
                    if layer == 0:
                        pq = p_q.tile([128, C9], f32, tag="pq")
                        for b in range(SB):
                            nc.tensor.matmul(
                                out=pq[:], lhsT=g_t[:, b * ROW:b * ROW + D],
                                rhs=s_t[:, b * C9:(b + 1) * C9],
                                start=(b == 0), stop=(b == SB - 1))
                    else:
                        pq = p_q.tile([128, CG], f32, tag="pq")
                        pg = p_q.tile([128, ND], f32, tag="m2", name="pg")
                        for b in range(SB):
                            nc.tensor.matmul(
                                out=pq[:], lhsT=g_t[:, b * ROW:b * ROW + D],
                                rhs=s_t[:, b * C9:b * C9 + CG],
                                start=(b == 0), stop=(b == SB - 1))
                        for b in range(SB):
                            nc.tensor.matmul(
                                out=pg[:], lhsT=g_t[:, b * ROW + D:b * ROW + 2 * D],
                                rhs=s_t[:, b * C9 + CG:(b + 1) * C9],
                                start=(b == 0), stop=(b == SB - 1))
                    ps = p_s.tile([1, C9], f32, tag="ps")
                    for b in range(SB):
                        nc.tensor.matmul(
                            out=ps[:], lhsT=ones_col[:],
                            rhs=s_t[:, b * C9:(b + 1) * C9],
                            start=(b == 0), stop=(b == SB - 1))

                    # ---- softmax/deg normalization (delayed)
                    smax = fp.tile([1, C9], f32, tag="smax")
                    nc.vector.tensor_scalar_max(smax[:], ps[:], 1e-3)
                    r_t = fp.tile([1, C9], f16, tag="r")
                    nc.vector.reciprocal(r_t[:], smax[:])
                    prb = p_rb.tile([128, C9], f32, tag="prb")
                    nc.tensor.matmul(out=prb[:], lhsT=ones_row[:], rhs=r_t[:],
                                     start=True, stop=True)
                    rb_sb = yp.tile([128, C9], f16, tag="rbsb")
                    nc.vector.tensor_copy(rb_sb[:], prb[:])
                    if layer == 0:
                        y_t = yp.tile([128, C9], f16, tag="y")
                        nc.vector.tensor_tensor(out=y_t[:], in0=pq[:], in1=rb_sb[:],
                                                op=OP.mult)
                        yq, yg = y_t[:, :CG], y_t[:, CG:C9]
                    else:
                        y_t = yp.tile([128, CG], f16, tag="y")
                        nc.vector.tensor_tensor(out=y_t[:], in0=pq[:],
                                                in1=rb_sb[:, :CG], op=OP.mult)
                        yg_t = yp.tile([128, ND], f16, tag="ygt")
                        nc.vector.tensor_tensor(out=yg_t[:], in0=pg[:],
                                                in1=rb_sb[:, CG:C9], op=OP.mult)
                        yq, yg = y_t[:], yg_t[:]

                    # ---- node-level weight application
                    if layer == 0:
                        F0 = D // H
                        # node-major mixes: out[j, h*16+f'] via lhsT=y-slice
                        x1p = p_m.tile([ND, D], f32, tag="m1", name="x1p")
                        for h in range(H):
                            nc.tensor.matmul(
                                out=x1p[:, h * F0:(h + 1) * F0],
                                lhsT=yq[:, h * ND:(h + 1) * ND],
                                rhs=w0c_t[:, h * F0:(h + 1) * F0],
                                start=True, stop=True)
                        x2p = p_m.tile([ND, D], f32, tag="m2", name="x2p")
                        nc.tensor.matmul(out=x2p[:], lhsT=yg, rhs=gw0_t[:],
                                         start=True, stop=True)
                        row_t = fp.tile([ND, ROW1], f16, tag="row")
                        # ELU(x1p) = relu + exp(min(,0)) - 1 -> row_t[:, 0:D]
                        rl = fp.tile([ND, D], f16, tag="rl")
                        nc.scalar.activation(rl[:], x1p[:], AF.Relu)
                        ng = fp.tile([ND, D], f32, tag="ng")
                        nc.vector.tensor_scalar_min(ng[:], x1p[:], 0.0)
                        em = fp.tile([ND, D], f16, tag="em")
                        nc.scalar.activation(em[:], ng[:], AF.Exp)
                        nc.vector.tensor_tensor(out=row_t[:, 0:D], in0=rl[:],
                                                in1=em[:], op=OP.add)
                        nc.vector.tensor_scalar_sub(row_t[:, 0:D], row_t[:, 0:D], 1.0)
                        nc.scalar.activation(row_t[:, D:2 * D], x2p[:], AF.Relu)
                        # es1/ed1 = x1f @ B1src/B1dst (needs feature-major x1f)
                        ptx = p_tr.tile([128, 256], f16, tag="tr2", name="ptx")[:D, :ND]
                        nc.tensor.transpose(ptx[:], row_t[:, 0:D], idn16[:ND, :ND])
                        xft = fp.tile([D, ND], f16, tag="xft")
                        nc.vector.tensor_copy(xft[:], ptx[:])
                        pe = p_tr.tile([128, 128], f32, tag="tr", name="pe")[:H, :2 * ND]
                        nc.tensor.matmul(out=pe[:, :ND], lhsT=b1s_t[:], rhs=xft[:],
                                         start=True, stop=True)
                        nc.tensor.matmul(out=pe[:, ND:], lhsT=b1d_t[:], rhs=xft[:],
                                         start=True, stop=True)
                        pe_sb = fp.tile([H, 2 * ND], f16, tag="pesb")
                        nc.vector.tensor_copy(pe_sb[:], pe[:])
                        ptr = p_tr.tile([128, 256], f16, tag="tr2", name="ptr")[:2 * ND, :H]
                        nc.tensor.transpose(ptr[:], pe_sb[:], idn16[0:H, 0:H])
                        esed = fp.tile([2 * ND, H], f16, tag="esed")
                        nc.vector.tensor_copy(esed[:], ptr[:])
                        nc.sync.dma_start(ed1_d[base:base + ND, :], esed[ND:, :])
                        nc.vector.tensor_copy(row_t[:, ESO1:ESO1 + H], esed[:ND, :])
                        nc.vector.memset(row_t[:, ESO1 + H:], 0.0)
                        nc.sync.dma_start(t1c[base:base + ND, :], row_t[:])
                    else:
                        o1p = p_m.tile([ND, D], f32, tag="m1", name="o1p")
                        for h in range(H):
                            nc.tensor.matmul(
                                out=o1p[:], lhsT=yq[:, h * ND:(h + 1) * ND],
                                rhs=w1c_t[:, h * D:(h + 1) * D],
                                start=(h == 0), stop=(h == H - 1))
                        o2p = p_m.tile([ND, D], f32, tag="m2", name="o2p")
                        nc.tensor.matmul(out=o2p[:], lhsT=yg, rhs=gw1_t[:],
                                         start=True, stop=True)
                        x1m = fp.tile([ND, D], f32, tag="x1m")
                        nc.scalar.activation(x1m[:], o1p[:], AF.Copy, scale=1.0 / H)
                        x2m = fp.tile([ND, D], f32, tag="x2m")
                        nc.scalar.activation(x2m[:], o2p[:], AF.Relu)
                        oo = fp.tile([ND, D], f32, tag="oo")
                        nc.vector.tensor_tensor(out=oo[:], in0=x1m[:], in1=x2m[:],
                                                op=OP.max)
                        nc.sync.dma_start(out_d[base:base + ND, :], oo[:])
                if layer == 0:
                    if NC > 1:
                        import concourse.mybir as mybir2
                        nc.gpsimd.collective_compute(
                            "AllGather", mybir2.AluOpType.bypass,
                            replica_groups=[list(range(NC))],
                            ins=[t1c[:]], outs=[t1f[:]])
                    else:
                        nc.sync.dma_start(t1f[:], t1c[:])
    nc.compile()
    return nc


# ------------------------------------------------------------------ runner
def _make_runner(nc, n_cores):
    import jax
    from jax.sharding import Mesh, PartitionSpec
    from jax.experimental.shard_map import shard_map
    import concourse.mybir as mybir
    from concourse.bass2jax import (_bass_exec_p, install_neuronx_cc_hook,
                                    partition_id_tensor)

    install_neuronx_cc_hook()
    partition_name = nc.partition_id_tensor.name if nc.partition_id_tensor else None
    in_names, out_names, out_avals = [], [], []
    for alloc in nc.m.functions[0].allocations:
        if not isinstance(alloc, mybir.MemoryLocationSet):
            continue
        name = alloc.memorylocations[0].name
        if alloc.kind == "ExternalInput":
            if name != partition_name:
                in_names.append(name)
        elif alloc.kind == "ExternalOutput":
            out_names.append(name)
            out_avals.append(jax.core.ShapedArray(
                tuple(alloc.tensor_shape), mybir.dt.np(alloc.dtype)))
    n_params, n_outs = len(in_names), len(out_avals)
    all_in = list(in_names) + list(out_names)
    if partition_name is not None:
        all_in.append(partition_name)

    def _body(*args):
        operands = list(args)
        if partition_name is not None:
            operands.append(partition_id_tensor())
        return tuple(_bass_exec_p.bind(
            *operands, out_avals=tuple(out_avals), in_names=tuple(all_in),
            out_names=tuple(out_names), lowering_input_output_aliases=(),
            sim_require_finite=True, sim_require_nnan=True, nc=nc))

    devices = jax.devices()[:n_cores]
    mesh = Mesh(np.asarray(devices), ("core",))
    sharded = jax.jit(
        shard_map(_body, mesh=mesh,
                  in_specs=(PartitionSpec("core"),) * (n_params + n_outs),
                  out_specs=(PartitionSpec("core"),) * n_outs, check_rep=False),
        donate_argnums=tuple(range(n_params, n_params + n_outs)), keep_unused=True)

    def run(in_maps):
        per_core = [[np.asarray(m[n]) for n in in_names] for m in in_maps]
        concat_in = [np.concatenate([per_core[c][i] for c in range(n_cores)], 0)
                     for i in range(n_params)]
        zeros = [np.zeros((n_cores * av.shape[0], *av.shape[1:]), av.dtype)
                 for av in out_avals]
        outs = sharded(*concat_in, *zeros)
        outs = [np.asarray(o) for o in outs]
        return [{n: outs[i].reshape(n_cores, *out_avals[i].shape)[c]
                 for i, n in enumerate(out_names)} for c in range(n_cores)]
    return run


def _prepare_inputs(cfg, inputs):
    pre = _preprocess(cfg, inputs["edge_index"], inputs["edge_weight"])
    hw = _host_weights(cfg, np.asarray(inputs["x"], np.float32),
                       *[np.asarray(inputs[k], np.float32) for k in
                         ("gat_W0", "gat_a0_src", "gat_a0_dst", "gat_W1",
                          "gat_a1_src", "gat_a1_dst", "gcn_W0", "gcn_W1")])
    NPCP = cfg["NPCP"]
    in_maps = []
    for c in range(cfg["NC"]):
        lo = c * NPCP
        ed0c = np.zeros((NPCP, cfg["H"]), np.float16)
        n_real = max(0, min(NPCP, cfg["N"] - lo))
        ed0c[:n_real] = hw["ed0"][lo:lo + n_real]
        in_maps.append(dict(
            table0=hw["table0"], idx_s=pre[c]["idx"], oh_s=pre[c]["oh"],
            ohT_s=pre[c]["ohT"], w_s=pre[c]["w"], ed0=ed0c,
            B1src=hw["B1src"], B1dst=hw["B1dst"], W0cat=hw["W0cat"],
            W1cat=hw["W1cat"], gcnW0=hw["gcn_W0"], gcnW1=hw["gcn_W1"]))
    return in_maps


def kernel(**inputs):
    cfg = _cfg()
    key = ("main", 1)
    if key not in _NC_CACHE:
        nc = _build(cfg, reps=1)
        _NC_CACHE[key] = _make_runner(nc, cfg["NC"])
    run = _NC_CACHE[key]
    in_maps = _prepare_inputs(cfg, inputs)
    res = run(in_maps)
    out = np.empty((cfg["N"], cfg["D"]), np.float32)
    NPCP = cfg["NPCP"]
    for c in range(cfg["NC"]):
        lo = c * NPCP
        n_real = max(0, min(NPCP, cfg["N"] - lo))
        out[lo:lo + n_real] = res[c]["out"][:n_real]
    return out


# revision 16
# speedup vs baseline: 8.1260x; 8.1260x over previous
"""Trainium2 Bass kernel for nn_GATmGCN (2-layer 8-head GAT + 2-layer weighted GCN,
output = elementwise max of the branches). 8-core SPMD, nodes sharded by dst range.

Design:
 - Host: sort edges by dst, shard nodes into contiguous per-core chunks, pack each
   core's edges into fixed tiles of ND=32 dst nodes with 3 A-table + 3 B-table
   128-slot gather blocks (node table split in half so int16 dma_gather indices fit).
 - Device per tile: dma_gather source rows [x|es] (fp16), one-hot matmuls accumulate
   unnormalized per-(head,node) sums + softmax denominators in PSUM (feature-major),
   normalization = reciprocal + ones-rank-1-matmul broadcast + multiply, then the
   weight matrices are applied at node level (aggregation/weight-mix commute).
 - Attention logits: es[src] rides in gathered rows; ed[dst] via one-hotT matmuls.
 - Layer 0 -> layer 1 node features exchanged with an AllGather.
"""
import numpy as np

_NC_CACHE = {}


def _cfg(N=50000, E=800000, n_cores=8):
    D, H, ND, ABLK, BBLK = 128, 8, 32, 3, 3
    SB = ABLK + BBLK
    t_global = (N + ND - 1) // ND
    T = (t_global + n_cores - 1) // n_cores          # tiles per core
    NPCP = T * ND                                     # padded nodes per core
    NPAD = NPCP * n_cores
    return dict(N=N, E=E, D=D, H=H, NC=n_cores, ND=ND, ABLK=ABLK, BBLK=BBLK,
                SB=SB, T=T, NPCP=NPCP, NPAD=NPAD, NHALF=NPAD // 2,
                ROW0=256, ROW1=384, ESO0=128, ESO1=256)


# ------------------------------------------------------------------ host side
def _wrap16(idx_flat):
    """[n] -> wrapped [16, n//16] (idx[i] = arr[i%16, i//16]) replicated to 128."""
    n = len(idx_flat)
    arr = idx_flat.astype(np.int16).reshape(n // 16, 16).T
    return np.tile(arr, (8, 1))


def _preprocess(cfg, edge_index, edge_weight):
    ND, ABLK, BBLK, SB = cfg["ND"], cfg["ABLK"], cfg["BBLK"], cfg["SB"]
    T, NPCP, NHALF, H = cfg["T"], cfg["NPCP"], cfg["NHALF"], cfg["H"]
    CAP_A, CAP_B = ABLK * 128, BBLK * 128

    src = np.asarray(edge_index[0], dtype=np.int64)
    dst = np.asarray(edge_index[1], dtype=np.int64)
    w = np.asarray(edge_weight, dtype=np.float32)
    order = np.argsort(dst, kind="stable")
    src, dst, w = src[order], dst[order], w[order]

    cores = []
    for c in range(cfg["NC"]):
        lo = c * NPCP
        a = np.searchsorted(dst, lo, "left")
        b = np.searchsorted(dst, lo + NPCP, "left")
        s_c, d_c, w_c = src[a:b], dst[a:b] - lo, w[a:b]
        tile = d_c // ND
        ldst = d_c - tile * ND
        isB = (s_c >= NHALF).astype(np.int64)
        # stable order by (tile, isB); edges already tile-sorted
        o2 = np.lexsort((np.arange(len(s_c)), isB, tile))
        tile, ldst, isB = tile[o2], ldst[o2], isB[o2]
        s2, w2 = s_c[o2], w_c[o2]
        # rank within (tile, isB) group
        gkey = tile * 2 + isB
        if len(gkey):
            starts = np.r_[0, np.flatnonzero(np.diff(gkey)) + 1]
            gid = np.zeros(len(gkey), np.int64)
            gid[starts[1:]] = 1
            gid = np.cumsum(gid)
            rank = np.arange(len(gkey)) - starts[gid]
            cnts = np.bincount(gkey, minlength=2 * T)
            assert cnts[0::2].max(initial=0) <= CAP_A, "A-run overflow"
            assert cnts[1::2].max(initial=0) <= CAP_B, "B-run overflow"
        else:
            rank = gkey
        slot = np.where(isB == 0, rank, CAP_A + rank)
        blk, part = slot // 128, slot % 128

        # wrapped gather indices (pad slots -> row 0)
        sidxA = np.zeros((T, CAP_A), np.int64)
        sidxB = np.zeros((T, CAP_B), np.int64)
        selA = isB == 0
        sidxA[tile[selA], rank[selA]] = s2[selA]
        sidxB[tile[~selA], rank[~selA]] = s2[~selA] - NHALF
        idx_s = np.zeros((T, 128, SB * 8), np.int16)
        wrapA = sidxA.reshape(T, CAP_A // 16, 16).transpose(0, 2, 1).astype(np.int16)
        wrapB = sidxB.reshape(T, CAP_B // 16, 16).transpose(0, 2, 1).astype(np.int16)
        idx_s[:, :, :ABLK * 8] = np.tile(wrapA, (1, 8, 1))
        idx_s[:, :, ABLK * 8:] = np.tile(wrapB, (1, 8, 1))

        oh_s = np.zeros((T, 128, SB * ND), np.float16)
        oh_s[tile, part, blk * ND + ldst] = 1.0
        ohT_s = np.zeros((T, ND, SB * 128), np.float16)
        ohT_s[tile, ldst, blk * 128 + part] = 1.0
        w_s = np.zeros((T, 128, SB), np.float16)
        w_s[tile, part, blk] = w2
        cores.append(dict(idx=idx_s, oh=oh_s, ohT=ohT_s, w=w_s))
    return cores


def _host_weights(cfg, x, gat_W0, gat_a0_src, gat_a0_dst, gat_W1, gat_a1_src,
                  gat_a1_dst, gcn_W0, gcn_W1):
    N, D, H, NPAD = cfg["N"], cfg["D"], cfg["H"], cfg["NPAD"]
    F0 = gat_W0.shape[2]
    B0src = np.einsum("hif,hf->ih", gat_W0, gat_a0_src).astype(np.float32)
    B0dst = np.einsum("hif,hf->ih", gat_W0, gat_a0_dst).astype(np.float32)
    B1src = np.einsum("hif,hf->ih", gat_W1, gat_a1_src).astype(np.float32)
    B1dst = np.einsum("hif,hf->ih", gat_W1, gat_a1_dst).astype(np.float32)
    es0 = x @ B0src
    ed0 = x @ B0dst
    W0cat = gat_W0.transpose(1, 0, 2).reshape(D, H * F0).astype(np.float32)
    W1cat = gat_W1.transpose(1, 0, 2).reshape(D, H * D).astype(np.float32)
    table0 = np.zeros((NPAD, cfg["ROW0"]), np.float16)
    table0[:N, :D] = x.astype(np.float16)
    table0[:N, cfg["ESO0"]:cfg["ESO0"] + H] = es0.astype(np.float16)
    return dict(table0=table0, ed0=ed0.astype(np.float16), B1src=B1src,
                B1dst=B1dst, W0cat=W0cat, W1cat=W1cat,
                gcn_W0=gcn_W0.astype(np.float32), gcn_W1=gcn_W1.astype(np.float32))


# ------------------------------------------------------------------ device side
def _build(cfg, reps=1, qrot=4):
    import concourse.bass as bass
    import concourse.tile as tile
    from concourse import bacc, mybir
    from concourse.masks import make_identity
    from contextlib import ExitStack

    f16, f32 = mybir.dt.float16, mybir.dt.float32
    i16 = mybir.dt.int16
    OP = mybir.AluOpType
    AF = mybir.ActivationFunctionType
    D, H, ND, SB, ABLK, BBLK = (cfg[k] for k in ("D", "H", "ND", "SB", "ABLK", "BBLK"))
    T, NPCP, NPAD, NHALF = (cfg[k] for k in ("T", "NPCP", "NPAD", "NHALF"))
    ROW0, ROW1, ESO0, ESO1 = (cfg[k] for k in ("ROW0", "ROW1", "ESO0", "ESO1"))
    NC = cfg["NC"]
    C9 = 9 * ND              # channels per sub-block in S (8 GAT + 1 GCN)
    CG = 8 * ND              # GAT channel count

    nc = bacc.Bacc("TRN2", target_bir_lowering=False, debug=False,
                   num_devices=NC, num_swdge_queues=4)

    t0 = nc.dram_tensor("table0", [NPAD, ROW0], f16, kind="ExternalInput").ap()
    idx_s = nc.dram_tensor("idx_s", [T, 128, SB * 8], i16, kind="ExternalInput").ap()
    oh_s = nc.dram_tensor("oh_s", [T, 128, SB * ND], f16, kind="ExternalInput").ap()
    ohT_s = nc.dram_tensor("ohT_s", [T, ND, SB * 128], f16, kind="ExternalInput").ap()
    w_s = nc.dram_tensor("w_s", [T, 128, SB], f16, kind="ExternalInput").ap()
    ed0_d = nc.dram_tensor("ed0", [NPCP, H], f16, kind="ExternalInput").ap()
    b1s_d = nc.dram_tensor("B1src", [D, H], f32, kind="ExternalInput").ap()
    b1d_d = nc.dram_tensor("B1dst", [D, H], f32, kind="ExternalInput").ap()
    w0c_d = nc.dram_tensor("W0cat", [D, D], f32, kind="ExternalInput").ap()
    w1c_d = nc.dram_tensor("W1cat", [D, H * D], f32, kind="ExternalInput").ap()
    gw0_d = nc.dram_tensor("gcnW0", [D, D], f32, kind="ExternalInput").ap()
    gw1_d = nc.dram_tensor("gcnW1", [D, D], f32, kind="ExternalInput").ap()
    out_d = nc.dram_tensor("out", [NPCP, D], f32, kind="ExternalOutput").ap()

    t1c = nc.dram_tensor("t1_chunk", [NPCP, ROW1], f16).ap()
    t1f = nc.dram_tensor("t1_full", [NPAD, ROW1], f16, addr_space="Shared").ap()
    ed1_d = nc.dram_tensor("ed1", [NPCP, H], f16).ap()

    with tile.TileContext(nc) as tc, ExitStack() as ctx, \
            nc.allow_low_precision(reason="fp16 edge pipeline by design"):
        const = ctx.enter_context(tc.tile_pool(name="const", bufs=1))
        ip = ctx.enter_context(tc.tile_pool(name="ip", bufs=2))
        gp = ctx.enter_context(tc.tile_pool(name="gp", bufs=3))
        hp = ctx.enter_context(tc.tile_pool(name="hp", bufs=2))
        sp = ctx.enter_context(tc.tile_pool(name="sp", bufs=2))
        yp = ctx.enter_context(tc.tile_pool(name="yp", bufs=2))
        fp = ctx.enter_context(tc.tile_pool(name="fp", bufs=2))
        pp = ctx.enter_context(tc.tile_pool(name="pp", bufs=1, space="PSUM"))
        p_ed = p_q = p_s = p_rb = p_m = p_tr = pp

        # constants
        ones_col = const.tile([128, 1], f16)
        nc.vector.memset(ones_col[:], 1.0)
        ones_row = const.tile([1, 128], f16)
        nc.vector.memset(ones_row[:], 1.0)
        idn16 = const.tile([128, 128], f16)
        make_identity(nc, idn16[:])
        idn32 = const.tile([128, 128], f32)
        make_identity(nc, idn32[:])
        w0c_t = const.tile([128, D], f16)
        nc.gpsimd.dma_start(w0c_t[:], w0c_d[:, :])
        w1c_t = const.tile([128, H * D], f16)
        nc.gpsimd.dma_start(w1c_t[:], w1c_d[:, :])
        gw0_t = const.tile([128, D], f16)
        nc.gpsimd.dma_start(gw0_t[:], gw0_d[:, :])
        gw1_t = const.tile([128, D], f16)
        nc.gpsimd.dma_start(gw1_t[:], gw1_d[:, :])
        b1s_t = const.tile([128, H], f16)
        nc.gpsimd.dma_start(b1s_t[:], b1s_d[:, :])
        b1d_t = const.tile([128, H], f16)
        nc.gpsimd.dma_start(b1d_t[:], b1d_d[:, :])

        for rep in range(reps):
            for layer in (0, 1):
                ROW = ROW0 if layer == 0 else ROW1
                ESO = ESO0 if layer == 0 else ESO1
                table = t0 if layer == 0 else t1f
                ed_src = ed0_d if layer == 0 else ed1_d

                for t in range(T):
                    base = t * ND
                    # ---- inputs for this tile
                    idx_t = ip.tile([128, SB * 8], i16, tag="idx")
                    nc.sync.dma_start(idx_t[:], idx_s[t])
                    oh_t = hp.tile([128, SB * ND], f16, tag="oh")
                    nc.sync.dma_start(oh_t[:], oh_s[t])
                    ohT_t = hp.tile([ND, SB * 128], f16, tag="ohT")
                    nc.sync.dma_start(ohT_t[:], ohT_s[t])
                    w_t = hp.tile([128, SB], f16, tag="w")
                    nc.sync.dma_start(w_t[:], w_s[t])
                    edT_t = hp.tile([ND, H], f16, tag="edT")
                    nc.sync.dma_start(edT_t[:], ed_src[base:base + ND, :])

                    # ---- gathers (A blocks then B blocks)
                    g_t = gp.tile([128, SB * ROW], f16, tag="g")
                    nc.gpsimd.dma_gather(
                        out_ap=g_t[:, :ABLK * ROW].rearrange("p (n e) -> p n e", e=ROW),
                        in_ap=table[0:NHALF, :],
                        idxs_ap=idx_t[:, :ABLK * 8],
                        num_idxs=ABLK * 128, num_idxs_reg=ABLK * 128,
                        elem_size=ROW, single_packet=False, queue_num=(2 * t) % qrot)
                    nc.gpsimd.dma_gather(
                        out_ap=g_t[:, ABLK * ROW:].rearrange("p (n e) -> p n e", e=ROW),
                        in_ap=table[NHALF:2 * NHALF, :],
                        idxs_ap=idx_t[:, ABLK * 8:],
                        num_idxs=BBLK * 128, num_idxs_reg=BBLK * 128,
                        elem_size=ROW, single_packet=False, queue_num=(2 * t + 1) % qrot)

                    # ---- attention logits z = exp(lrelu(es[src] + ed[dst]))
                    ped = p_ed.tile([128, SB * H], f32, tag="ped")
                    for b in range(SB):
                        nc.tensor.matmul(
                            out=ped[:, b * H:(b + 1) * H],
                            lhsT=ohT_t[:, b * 128:(b + 1) * 128],
                            rhs=edT_t[:], start=True, stop=True)
                    es_view = g_t[:].rearrange("p (b e) -> p b e", e=ROW)[:, :, ESO:ESO + H]
                    u_t = sp.tile([128, SB * H], f16, tag="u")
                    nc.vector.tensor_tensor(
                        out=u_t[:], in0=ped[:], in1=es_view, op=OP.add)
                    lr_t = sp.tile([128, SB * H], f16, tag="lr")
                    nc.vector.tensor_scalar_mul(lr_t[:], u_t[:], 0.2)
                    nc.vector.tensor_tensor(out=lr_t[:], in0=u_t[:], in1=lr_t[:],
                                            op=OP.max)
                    attw = sp.tile([128, SB * 9], f16, tag="attw")
                    av = attw[:].rearrange("p (b c) -> p b c", c=9)
                    nc.scalar.activation(
                        av[:, :, 0:H],
                        lr_t[:].rearrange("p (b c) -> p b c", c=H), AF.Exp)
                    nc.vector.tensor_copy(av[:, :, H:9], w_t[:, :, None])

                    # ---- S = attw (x) onehot   [128, SB*9*ND]
                    s_t = sp.tile([128, SB * C9], f16, tag="s")
                    sv = s_t[:].rearrange("p (b c j) -> p b c j", c=9, j=ND)
                    ohv = oh_t[:].rearrange("p (b j) -> p b j", j=ND)
                    nc.vector.tensor_tensor(
                        out=sv,
                        in0=ohv[:, :, None, :].to_broadcast((128, SB, 9, ND)),
                        in1=av[:, :, :, None].to_broadcast((128, SB, 9, ND)),
                        op=OP.mult)

                    # ---- aggregation matmuls
                    if layer == 0:
                        pq = p_q.tile([128, C9], f32, tag="pq")
                        for b in range(SB):
                            nc.tensor.matmul(
                                out=pq[:], lhsT=g_t[:, b * ROW:b * ROW + D],
                                rhs=s_t[:, b * C9:(b + 1) * C9],
                                start=(b == 0), stop=(b == SB - 1))
                    else:
                        pq = p_q.tile([128, CG], f32, tag="pq")
                        pg = p_q.tile([128, ND], f32, tag="m2", name="pg")
                        for b in range(SB):
                            nc.tensor.matmul(
                                out=pq[:], lhsT=g_t[:, b * ROW:b * ROW + D],
                                rhs=s_t[:, b * C9:b * C9 + CG],
                                start=(b == 0), stop=(b == SB - 1))
                        for b in range(SB):
                            nc.tensor.matmul(
                                out=pg[:], lhsT=g_t[:, b * ROW + D:b * ROW + 2 * D],
                                rhs=s_t[:, b * C9 + CG:(b + 1) * C9],
                                start=(b == 0), stop=(b == SB - 1))
                    ps = p_s.tile([1, C9], f32, tag="ps")
                    for b in range(SB):
                        nc.tensor.matmul(
                            out=ps[:], lhsT=ones_col[:],
                            rhs=s_t[:, b * C9:(b + 1) * C9],
                            start=(b == 0), stop=(b == SB - 1))

                    # ---- softmax/deg normalization (delayed)
                    smax = fp.tile([1, C9], f32, tag="smax")
                    nc.vector.tensor_scalar_max(smax[:], ps[:], 1e-3)
                    r_t = fp.tile([1, C9], f16, tag="r")
                    nc.vector.reciprocal(r_t[:], smax[:])
                    prb = p_rb.tile([128, C9], f32, tag="prb")
                    nc.tensor.matmul(out=prb[:], lhsT=ones_row[:], rhs=r_t[:],
                                     start=True, stop=True)
                    rb_sb = yp.tile([128, C9], f16, tag="rbsb")
                    nc.vector.tensor_copy(rb_sb[:], prb[:])
                    if layer == 0:
                        y_t = yp.tile([128, C9], f16, tag="y")
                        nc.vector.tensor_tensor(out=y_t[:], in0=pq[:], in1=rb_sb[:],
                                                op=OP.mult)
                        yq, yg = y_t[:, :CG], y_t[:, CG:C9]
                    else:
                        y_t = yp.tile([128, CG], f16, tag="y")
                        nc.vector.tensor_tensor(out=y_t[:], in0=pq[:],
                                                in1=rb_sb[:, :CG], op=OP.mult)
                        yg_t = yp.tile([128, ND], f16, tag="ygt")
                        nc.vector.tensor_tensor(out=yg_t[:], in0=pg[:],
                                                in1=rb_sb[:, CG:C9], op=OP.mult)
                        yq, yg = y_t[:], yg_t[:]

                    # ---- node-level weight application
                    if layer == 0:
                        F0 = D // H
                        # node-major mixes: out[j, h*16+f'] via lhsT=y-slice
                        x1p = p_m.tile([ND, D], f32, tag="m1", name="x1p")
                        for h in range(H):
                            nc.tensor.matmul(
                                out=x1p[:, h * F0:(h + 1) * F0],
                                lhsT=yq[:, h * ND:(h + 1) * ND],
                                rhs=w0c_t[:, h * F0:(h + 1) * F0],
                                start=True, stop=True)
                        x2p = p_m.tile([ND, D], f32, tag="m2", name="x2p")
                        nc.tensor.matmul(out=x2p[:], lhsT=yg, rhs=gw0_t[:],
                                         start=True, stop=True)
                        row_t = fp.tile([ND, ROW1], f16, tag="row")
                        # ELU(x1p) = relu + exp(min(,0)) - 1 -> row_t[:, 0:D]
                        rl = fp.tile([ND, D], f16, tag="rl")
                        nc.scalar.activation(rl[:], x1p[:], AF.Relu)
                        ng = fp.tile([ND, D], f32, tag="ng")
                        nc.vector.tensor_scalar_min(ng[:], x1p[:], 0.0)
                        em = fp.tile([ND, D], f16, tag="em")
                        nc.scalar.activation(em[:], ng[:], AF.Exp)
                        nc.vector.tensor_tensor(out=row_t[:, 0:D], in0=rl[:],
                                                in1=em[:], op=OP.add)
                        nc.vector.tensor_scalar_sub(row_t[:, 0:D], row_t[:, 0:D], 1.0)
                        nc.scalar.activation(row_t[:, D:2 * D], x2p[:], AF.Relu)
                        # es1/ed1 = x1f @ B1src/B1dst (needs feature-major x1f)
                        ptx = p_tr.tile([128, 256], f16, tag="tr2", name="ptx")[:D, :ND]
                        nc.tensor.transpose(ptx[:], row_t[:, 0:D], idn16[:ND, :ND])
                        xft = fp.tile([D, ND], f16, tag="xft")
                        nc.vector.tensor_copy(xft[:], ptx[:])
                        pe = p_tr.tile([128, 128], f32, tag="tr", name="pe")[:H, :2 * ND]
                        nc.tensor.matmul(out=pe[:, :ND], lhsT=b1s_t[:], rhs=xft[:],
                                         start=True, stop=True)
                        nc.tensor.matmul(out=pe[:, ND:], lhsT=b1d_t[:], rhs=xft[:],
                                         start=True, stop=True)
                        pe_sb = fp.tile([H, 2 * ND], f16, tag="pesb")
                        nc.vector.tensor_copy(pe_sb[:], pe[:])
                        ptr = p_tr.tile([128, 256], f16, tag="tr2", name="ptr")[:2 * ND, :H]
                        nc.tensor.transpose(ptr[:], pe_sb[:], idn16[0:H, 0:H])
                        esed = fp.tile([2 * ND, H], f16, tag="esed")
                        nc.vector.tensor_copy(esed[:], ptr[:])
                        nc.sync.dma_start(ed1_d[base:base + ND, :], esed[ND:, :])
                        nc.vector.tensor_copy(row_t[:, ESO1:ESO1 + H], esed[:ND, :])
                        nc.vector.memset(row_t[:, ESO1 + H:], 0.0)
                        nc.sync.dma_start(t1c[base:base + ND, :], row_t[:])
                    else:
                        o1p = p_m.tile([ND, D], f32, tag="m1", name="o1p")
                        for h in range(H):
                            nc.tensor.matmul(
                                out=o1p[:], lhsT=yq[:, h * ND:(h + 1) * ND],
                                rhs=w1c_t[:, h * D:(h + 1) * D],
                                start=(h == 0), stop=(h == H - 1))
                        o2p = p_m.tile([ND, D], f32, tag="m2", name="o2p")
                        nc.tensor.matmul(out=o2p[:], lhsT=yg, rhs=gw1_t[:],
                                         start=True, stop=True)
                        x1m = fp.tile([ND, D], f32, tag="x1m")
                        nc.scalar.activation(x1m[:], o1p[:], AF.Copy, scale=1.0 / H)
                        x2m = fp.tile([ND, D], f32, tag="x2m")
                        nc.scalar.activation(x2m[:], o2p[:], AF.Relu)
                        oo = fp.tile([ND, D], f32, tag="oo")
                        nc.vector.tensor_tensor(out=oo[:], in0=x1m[:], in1=x2m[:],
                                                op=OP.max)
                        nc.sync.dma_start(out_d[base:base + ND, :], oo[:])
                if layer == 0:
                    if NC > 1:
                        import concourse.mybir as mybir2
                        nc.gpsimd.collective_compute(
                            "AllGather", mybir2.AluOpType.bypass,
                            replica_groups=[list(range(NC))],
                            ins=[t1c[:]], outs=[t1f[:]])
                    else:
                        nc.sync.dma_start(t1f[:], t1c[:])
    nc.compile()
    return nc


# ------------------------------------------------------------------ runner
def _make_runner(nc, n_cores):
    import jax
    from jax.sharding import Mesh, PartitionSpec
    from jax.experimental.shard_map import shard_map
    import concourse.mybir as mybir
    from concourse.bass2jax import (_bass_exec_p, install_neuronx_cc_hook,
                                    partition_id_tensor)

    install_neuronx_cc_hook()
    partition_name = nc.partition_id_tensor.name if nc.partition_id_tensor else None
    in_names, out_names, out_avals = [], [], []
    for alloc in nc.m.functions[0].allocations:
        if not isinstance(alloc, mybir.MemoryLocationSet):
            continue
        name = alloc.memorylocations[0].name
        if alloc.kind == "ExternalInput":
            if name != partition_name:
                in_names.append(name)
        elif alloc.kind == "ExternalOutput":
            out_names.append(name)
            out_avals.append(jax.core.ShapedArray(
                tuple(alloc.tensor_shape), mybir.dt.np(alloc.dtype)))
    n_params, n_outs = len(in_names), len(out_avals)
    all_in = list(in_names) + list(out_names)
    if partition_name is not None:
        all_in.append(partition_name)

    def _body(*args):
        operands = list(args)
        if partition_name is not None:
            operands.append(partition_id_tensor())
        return tuple(_bass_exec_p.bind(
            *operands, out_avals=tuple(out_avals), in_names=tuple(all_in),
            out_names=tuple(out_names), lowering_input_output_aliases=(),
            sim_require_finite=True, sim_require_nnan=True, nc=nc))

    devices = jax.devices()[:n_cores]
    mesh = Mesh(np.asarray(devices), ("core",))
    sharded = jax.jit(
        shard_map(_body, mesh=mesh,
                  in_specs=(PartitionSpec("core"),) * (n_params + n_outs),
                  out_specs=(PartitionSpec("core"),) * n_outs, check_rep=False),
        donate_argnums=tuple(range(n_params, n_params + n_outs)), keep_unused=True)

    def put_inputs(in_maps):
        from jax.sharding import NamedSharding
        sh = NamedSharding(mesh, PartitionSpec("core"))
        per_core = [[np.asarray(m[n]) for n in in_names] for m in in_maps]
        concat_in = [np.concatenate([per_core[c][i] for c in range(n_cores)], 0)
                     for i in range(n_params)]
        return [jax.device_put(a, sh) for a in concat_in]

    def run_dev(dev_in):
        zeros = [np.zeros((n_cores * av.shape[0], *av.shape[1:]), av.dtype)
                 for av in out_avals]
        outs = sharded(*dev_in, *zeros)
        outs = [np.asarray(o) for o in outs]
        return [{n: outs[i].reshape(n_cores, *out_avals[i].shape)[c]
                 for i, n in enumerate(out_names)} for c in range(n_cores)]

    def run(in_maps):
        return run_dev(put_inputs(in_maps))

    run.put_inputs = put_inputs
    run.run_dev = run_dev
    return run


def _prepare_inputs(cfg, inputs):
    pre = _preprocess(cfg, inputs["edge_index"], inputs["edge_weight"])
    hw = _host_weights(cfg, np.asarray(inputs["x"], np.float32),
                       *[np.asarray(inputs[k], np.float32) for k in
                         ("gat_W0", "gat_a0_src", "gat_a0_dst", "gat_W1",
                          "gat_a1_src", "gat_a1_dst", "gcn_W0", "gcn_W1")])
    NPCP = cfg["NPCP"]
    in_maps = []
    for c in range(cfg["NC"]):
        lo = c * NPCP
        ed0c = np.zeros((NPCP, cfg["H"]), np.float16)
        n_real = max(0, min(NPCP, cfg["N"] - lo))
        ed0c[:n_real] = hw["ed0"][lo:lo + n_real]
        in_maps.append(dict(
            table0=hw["table0"], idx_s=pre[c]["idx"], oh_s=pre[c]["oh"],
            ohT_s=pre[c]["ohT"], w_s=pre[c]["w"], ed0=ed0c,
            B1src=hw["B1src"], B1dst=hw["B1dst"], W0cat=hw["W0cat"],
            W1cat=hw["W1cat"], gcnW0=hw["gcn_W0"], gcnW1=hw["gcn_W1"]))
    return in_maps


def kernel(**inputs):
    cfg = _cfg()
    key = ("main", 1)
    if key not in _NC_CACHE:
        nc = _build(cfg, reps=1)
        _NC_CACHE[key] = _make_runner(nc, cfg["NC"])
    run = _NC_CACHE[key]
    in_maps = _prepare_inputs(cfg, inputs)
    res = run(in_maps)
    out = np.empty((cfg["N"], cfg["D"]), np.float32)
    NPCP = cfg["NPCP"]
    for c in range(cfg["NC"]):
        lo = c * NPCP
        n_real = max(0, min(NPCP, cfg["N"] - lo))
        out[lo:lo + n_real] = res[c]["out"][:n_real]
    return out
